# revision 16
# baseline (speedup 1.0000x reference)
"""Multi-head self-attention block on Trainium2, 8-core SPMD.

Problem (fixed shapes): x(2,2048,1024), causal-additive mask(2,2048,2048),
w_qkv(1024,3072), b_qkv(3072), w_out(1024,1024), b_out(1024).
out = MHSA(x) with H=16 heads, head_dim=64.

Sharding strategy:
  - QKV projection + attention: tensor-parallel over heads (2 heads/core).
    Each core computes Q^T,K^T,V for its 2 heads over all 4096 tokens.
  - All matmul operands are bf16 (inputs converted on host); PSUM
    accumulation stays fp32, so softmax denominators are exact sums of
    bf16-rounded exp terms. Scores are computed transposed ([keys, q])
    so softmax needs no transposes: exp on ScalarE over [128, 1024]
    pairs of key-blocks (halves ACTIVATE dispatch overhead), the
    denominator comes from an all-ones column appended to V (fused into
    the AV matmul), normalization is DVE reciprocal + GpSimd
    partition_broadcast + one DVE multiply.
  - Softmax skips max-subtraction: scores here are O(10) and exp() runs
    in fp32; masked lanes are exactly zero, matching exp(-1e9)==0.
  - Attention runs as two head-passes; each pass ends in a bf16
    AllToAll of its 64 attention features, so the first collective
    hides under the second pass's compute.
  - After the AllToAlls each core holds all 1024 attention features for
    its 512 tokens and runs the out projection (+bias via a rank-1
    matmul) for just those rows. Host concatenates the 8 row-blocks.
"""

import os
import sys
from contextlib import ExitStack

if "/opt/trn_rl_repo" not in sys.path:
    sys.path.insert(0, "/opt/trn_rl_repo")

import numpy as np

import concourse.mybir as mybir
import concourse.tile as tile
from concourse import bacc, bass_utils

B, S, D, H, HD = 2, 2048, 1024, 16, 64
NCORES = 8
SL = B * S            # 4096 tokens total
QC = 512              # q-chunk / moving free dim
KC = 128              # k-chunk (partition dim)
NQ = S // QC          # 4 q-chunks per batch
NK = S // KC          # 16 k-chunks per batch
NT = SL // QC         # 8 token chunks
DK = D // 128         # 8 contraction chunks of the model dim
VW = 2 * (HD + 1)     # 130: V-natural block width (2 heads x (64 V + ones col))

f32 = mybir.dt.float32
bf16 = mybir.dt.bfloat16
FX = mybir.ActivationFunctionType
ALU = mybir.AluOpType

LAST_EXEC_NS = None   # HW exec time (ns) of the last traced run
LAST_RESULTS = None


def _build(variant, exp_bias=0.0):
    """Emit the SPMD program. variant: 'causal' | 'dense' | 'general'."""
    assert variant in ("causal", "dense", "general")
    nc = bacc.Bacc("TRN2", target_bir_lowering=False, debug=False,
                   num_devices=NCORES)

    xT_d = nc.dram_tensor("xT", [D, SL], bf16, kind="ExternalInput")
    wqkv_d = nc.dram_tensor("wqkv", [D, 384], bf16, kind="ExternalInput")
    bqkv_d = nc.dram_tensor("bqkv", [128, 3], f32, kind="ExternalInput")
    wout_d = nc.dram_tensor("wout", [D, D], bf16, kind="ExternalInput")
    bout_d = nc.dram_tensor("bout", [1, D], bf16, kind="ExternalInput")
    ident_d = nc.dram_tensor("ident", [128, 128], bf16, kind="ExternalInput")
    if variant == "causal":
        # 0/1 multiplicative causal mask for the 4 diagonal sub-blocks,
        # applied to exp(scores) (exp(s-1e9)==0 == exp(s)*0).
        maskz_d = nc.dram_tensor("maskz", [128, 4 * QC], bf16, kind="ExternalInput")
    if variant == "general":
        maskT_d = nc.dram_tensor("maskT", [B, S, S], f32, kind="ExternalInput")
    out_d = nc.dram_tensor("out", [QC, D], f32, kind="ExternalOutput")

    with tile.TileContext(nc) as tc:
        with ExitStack() as stack:
            ep = stack.enter_context
            cpool = ep(tc.tile_pool(name="consts", bufs=1))
            big = ep(tc.tile_pool(name="big", bufs=1))
            xpool = ep(tc.tile_pool(name="xts", bufs=16))
            vpool = ep(tc.tile_pool(name="vstg", bufs=2))
            epool = ep(tc.tile_pool(name="epool", bufs=4))
            mpool = ep(tc.tile_pool(name="mpool", bufs=4))
            rpool = ep(tc.tile_pool(name="rpool", bufs=2))
            bcpool = ep(tc.tile_pool(name="bcpool", bufs=2))
            apool = ep(tc.tile_pool(name="apool", bufs=2))
            ppool = ep(tc.tile_pool(name="ppool", bufs=16))
            opool = ep(tc.tile_pool(name="opool", bufs=2))
            dram = ep(tc.tile_pool(name="dram", bufs=1, space="DRAM"))
            psmm = ep(tc.tile_pool(name="psmm", bufs=2, space="PSUM"))
            pssc = ep(tc.tile_pool(name="pssc", bufs=2, space="PSUM"))
            psav = ep(tc.tile_pool(name="psav", bufs=2, space="PSUM"))

            # ---------------- constants / resident tensors ----------------
            # w chunks are fetched interleaved with the first x chunks (see
            # qkv_gen) so the first matmul waits on just two transfers.
            w_sb = big.tile([128, DK * 384], bf16, name="w_sb")
            bq_sb = cpool.tile([128, 3], f32, name="bq_sb")
            ident = cpool.tile([128, 128], bf16, name="ident")
            if variant == "causal":
                maskz_sb = cpool.tile([128, 4 * QC], bf16, name="maskz_sb")
            ones_bf = cpool.tile([1, 128], bf16, name="ones_bf")
            nc.vector.memset(ones_bf[:], 1.0)

            wo_sb = big.tile([128, DK * D], bf16, name="wo_sb")
            bo_sb = cpool.tile([1, D], bf16, name="bo_sb")

            qT = big.tile([128, SL], bf16, name="qT")
            kT = big.tile([128, SL], bf16, name="kT")
            vn = big.tile([128, B * NK * VW], bf16, name="vn")
            # ones columns for the softmax denominator live at 64 + 65*j
            vn_ones = vn[:].rearrange("p (b c) -> p b c", c=HD + 1)[:, :, 64:65]
            nc.vector.memset(vn_ones, 1.0)

            a2a_in = [dram.tile([NCORES, HD, QC], bf16, name=f"a2a_in{h}")
                      for h in range(2)]
            a2a_out = [dram.tile([NCORES, HD, QC], bf16, name=f"a2a_out{h}")
                       for h in range(2)]

            # ---------------- phase 1: QKV projection (generator) ----------
            def qkv_gen(t):
                xts = []
                for dk in range(DK):
                    if t == 0:
                        nc.sync.dma_start(
                            w_sb[:, 384 * dk:384 * (dk + 1)],
                            wqkv_d.ap()[128 * dk:128 * (dk + 1), :])
                    xt = xpool.tile([128, QC], bf16, name=f"xt{t}_{dk}", tag="xt")
                    nc.sync.dma_start(
                        xt[:], xT_d.ap()[128 * dk:128 * (dk + 1),
                                         QC * t:QC * (t + 1)])
                    xts.append(xt)
                if t == 0:
                    nc.sync.dma_start(bq_sb[:], bqkv_d.ap())
                    nc.sync.dma_start(ident[:], ident_d.ap())
                    if variant == "causal":
                        nc.sync.dma_start(maskz_sb[:], maskz_d.ap())
                for m in range(3):
                    ps = psmm.tile([128, QC], f32, name=f"qkv{t}_{m}", tag="mm")
                    for dk in range(DK):
                        c0 = 384 * dk + 128 * m
                        nc.tensor.matmul(ps[:],
                                         w_sb[:, c0:c0 + 128],
                                         xts[dk][:],
                                         start=(dk == 0), stop=(dk == DK - 1))
                    bias_ap = bq_sb[:, m:m + 1]
                    if m == 0:
                        nc.vector.tensor_scalar_add(
                            out=qT[:, QC * t:QC * (t + 1)], in0=ps[:], scalar1=bias_ap)
                    elif m == 1:
                        nc.vector.tensor_scalar_add(
                            out=kT[:, QC * t:QC * (t + 1)], in0=ps[:], scalar1=bias_ap)
                    else:
                        vst = vpool.tile([128, QC], bf16, name=f"vst{t}", tag="vst")
                        nc.vector.tensor_scalar_add(out=vst[:], in0=ps[:], scalar1=bias_ap)
                        for ci in range(4):
                            gi = 4 * t + ci
                            trp = psmm.tile([128, 128], bf16, name=f"tr{gi}", tag="mm")
                            nc.tensor.transpose(trp[:], vst[:, 128 * ci:128 * (ci + 1)],
                                                ident[:])
                            nc.vector.tensor_copy(
                                out=vn[:, VW * gi:VW * gi + 64], in_=trp[:, 0:64])
                            nc.vector.tensor_copy(
                                out=vn[:, VW * gi + 65:VW * gi + 129], in_=trp[:, 64:128])
                    yield

            # ---------------- phase 2: attention (generator, one head) -----
            # pend holds at most one pair awaiting its AV matmuls so the AV
            # for pair p issues only after exp(p+1) is queued (PE never
            # waits on ScalarE back-to-back).
            pend = []

            def drain_pend():
                e, av, gis, h, st, sp_, fin = pend.pop(0)
                for z, gi in enumerate(gis):
                    nc.tensor.matmul(av[:],
                                     vn[:, VW * gi + 65 * h:VW * gi + 65 * h + 65],
                                     e[:, QC * z:QC * (z + 1)],
                                     start=(st and z == 0), stop=(sp_ and z == 1),
                                     skip_group_check=True)
                if fin is not None:
                    fin()

            def attn_gen(h, b, j):
                n_i = 4 * (j + 1) if variant == "causal" else NK
                npair = n_i // 2
                q0 = S * b + QC * j
                h0 = 64 * h
                av = psav.tile([65, QC], f32, name=f"av{h}_{b}_{j}", tag="av")

                def finalize():
                    dd = rpool.tile([1, QC], f32, name=f"dd{h}_{b}_{j}", tag="dd")
                    nc.vector.tensor_copy(out=dd[:], in_=av[64:65, :])
                    rr = rpool.tile([1, QC], f32, name=f"rr{h}_{b}_{j}", tag="rr")
                    nc.vector.reciprocal_approx_fast(out=rr[:], in_=dd[:])
                    bc = bcpool.tile([64, QC], f32, name=f"bc{h}_{b}_{j}", tag="bc")
                    nc.gpsimd.partition_broadcast(bc[:], rr[:], channels=64)
                    att = apool.tile([64, QC], bf16, name=f"att{h}_{b}_{j}", tag="att")
                    nc.vector.tensor_tensor(out=att[:], in0=av[0:64, :], in1=bc[:],
                                            op=ALU.mult)
                    nc.sync.dma_start(a2a_in[h][NQ * b + j], att[:])

                for p in range(npair):
                    i0, i1 = 2 * p, 2 * p + 1
                    gi0, gi1 = NK * b + i0, NK * b + i1
                    k0 = S * b + KC * i0
                    sp = pssc.tile([128, 2 * QC], f32, name=f"s{h}_{b}_{j}_{p}",
                                   tag="sc")
                    nc.tensor.matmul(sp[:, 0:QC], kT[h0:h0 + 64, k0:k0 + KC],
                                     qT[h0:h0 + 64, q0:q0 + QC],
                                     start=True, stop=True)
                    nc.tensor.matmul(sp[:, QC:2 * QC],
                                     kT[h0:h0 + 64, k0 + KC:k0 + 2 * KC],
                                     qT[h0:h0 + 64, q0:q0 + QC],
                                     start=True, stop=True)
                    if variant == "general":
                        mt = mpool.tile([128, 2 * QC], f32,
                                        name=f"mt{h}_{b}_{j}_{p}", tag="mt")
                        for z, ii in enumerate((i0, i1)):
                            nc.sync.dma_start(
                                mt[:, QC * z:QC * (z + 1)],
                                maskT_d.ap()[b, KC * ii:KC * (ii + 1),
                                             QC * j:QC * (j + 1)])
                        nc.vector.tensor_tensor(out=sp[:], in0=sp[:], in1=mt[:],
                                                op=ALU.add)
                    e = epool.tile([128, 2 * QC], bf16, name=f"e{h}_{b}_{j}_{p}",
                                   tag="e")
                    nc.scalar.activation(out=e[:], in_=sp[:], func=FX.Exp,
                                         bias=exp_bias)
                    if variant == "causal" and p >= npair - 2:
                        dp = p - (npair - 2)  # 0 or 1 -> mask cols m=0,1 / 2,3
                        nc.vector.tensor_tensor(
                            out=e[:], in0=e[:],
                            in1=maskz_sb[:, 2 * QC * dp:2 * QC * (dp + 1)],
                            op=ALU.mult)
                    pend.append((e, av, (gi0, gi1), h,
                                 p == 0, p == npair - 1,
                                 finalize if p == npair - 1 else None))
                    if len(pend) > 1:
                        drain_pend()
                    yield

            def run_all(gens):
                gens = list(gens)
                while gens:
                    nxt = []
                    for g in gens:
                        try:
                            next(g)
                            nxt.append(g)
                        except StopIteration:
                            pass
                    gens = nxt

            # ----- interleave qkv t-chunks with head-0 attention -----------
            blocks = [(b, j) for b in range(B) for j in range(NQ)]
            run_all([qkv_gen(0)])
            # out-projection weights trickle in behind the x chunks
            for dk in range(DK):
                nc.sync.dma_start(wo_sb[:, D * dk:D * (dk + 1)],
                                  wout_d.ap()[128 * dk:128 * (dk + 1), :])
            nc.sync.dma_start(bo_sb[:], bout_d.ap())
            for t in range(1, NT):
                b, j = blocks[t - 1]
                run_all([qkv_gen(t), attn_gen(0, b, j)])
            run_all([attn_gen(0, *blocks[NT - 1])])
            while pend:
                drain_pend()

            # ---------------- phase 3: AllToAll h0 + head-1 pass -----------
            nc.gpsimd.collective_compute(
                "AllToAll", ALU.bypass,
                replica_groups=[list(range(NCORES))],
                ins=[a2a_in[0].opt()], outs=[a2a_out[0].opt()])

            for b, j in blocks:
                run_all([attn_gen(1, b, j)])
            while pend:
                drain_pend()

            nc.gpsimd.collective_compute(
                "AllToAll", ALU.bypass,
                replica_groups=[list(range(NCORES))],
                ins=[a2a_in[1].opt()], outs=[a2a_out[1].opt()])

            # ---------------- phase 4: out projection ----------------------
            # all received-attention tiles load up front (they fire the
            # moment the AllToAll lands), then the matmuls stream.
            atw = []
            for dk in range(DK):
                at = ppool.tile([128, QC], bf16, name=f"atw{dk}", tag="at")
                for h in range(2):
                    nc.sync.dma_start(at[64 * h:64 * (h + 1), :],
                                      a2a_out[h][dk])
                atw.append(at)
            for qsub in range(4):
                for dc in range(2):
                    ps = psmm.tile([128, QC], f32, name=f"op{qsub}_{dc}", tag="mm")
                    for dk in range(DK):
                        c0 = D * dk + QC * dc
                        nc.tensor.matmul(ps[:],
                                         atw[dk][:, 128 * qsub:128 * (qsub + 1)],
                                         wo_sb[:, c0:c0 + QC],
                                         start=(dk == 0), stop=False)
                    nc.tensor.matmul(ps[:], ones_bf[:],
                                     bo_sb[0:1, QC * dc:QC * (dc + 1)],
                                     start=False, stop=True)
                    osb = opool.tile([128, QC], f32, name=f"osb{qsub}_{dc}", tag="osb")
                    nc.vector.tensor_copy(out=osb[:], in_=ps[:])
                    nc.sync.dma_start(
                        out_d.ap()[128 * qsub:128 * (qsub + 1),
                                   QC * dc:QC * (dc + 1)], osb[:])

    nc.finalize()
    return nc


def _detect_variant(mask):
    if not mask.any():
        return "dense"
    tri = np.where(np.tril(np.ones((S, S), dtype=bool)),
                   np.float32(0.0), np.float32(-1e9)).astype(np.float32)
    for b in range(B):
        if not np.array_equal(mask[b], tri):
            return "general"
    return "causal"


def kernel(**inputs):
    global LAST_EXEC_NS, LAST_RESULTS
    import ml_dtypes
    bf = ml_dtypes.bfloat16

    x = np.ascontiguousarray(np.asarray(inputs["x"], dtype=np.float32))
    mask = np.asarray(inputs["mask"], dtype=np.float32)
    w_qkv = np.asarray(inputs["w_qkv"], dtype=np.float32)
    b_qkv = np.asarray(inputs["b_qkv"], dtype=np.float32)
    w_out = np.ascontiguousarray(np.asarray(inputs["w_out"], dtype=np.float32))
    b_out = np.asarray(inputs["b_out"], dtype=np.float32)

    variant = _detect_variant(mask)

    exp_bias = 0.0
    maskT = None
    if variant in ("general", "dense"):
        # guard exp() against overflow: bound max score via norms; any
        # needed shift is folded into the (transposed) additive mask.
        xf = x.reshape(SL, D)
        qkv = xf @ w_qkv + b_qkv
        qkv = qkv.reshape(SL, H, 3 * HD)
        qn = np.linalg.norm(qkv[:, :, :HD], axis=2).max()
        kn = np.linalg.norm(qkv[:, :, HD:2 * HD], axis=2).max()
        mmax = 0.0 if variant == "dense" else max(0.0, float(np.nanmax(mask)))
        bound = qn * kn / np.sqrt(HD) + mmax
        shift = min(0.0, 60.0 - bound)
        if variant == "dense":
            exp_bias = shift
        if variant == "general":
            maskT = np.ascontiguousarray(
                mask.transpose(0, 2, 1) + np.float32(shift))

    xT = np.ascontiguousarray(x.reshape(SL, D).T.astype(bf))
    const_ident = np.eye(128, dtype=bf)
    const_maskz = None
    if variant == "causal":
        const_maskz = np.zeros((128, 4 * QC), dtype=bf)
        for m in range(4):
            dk = np.arange(128)[:, None]
            dq = np.arange(QC)[None, :]
            const_maskz[:, QC * m:QC * (m + 1)] = (
                128 * m + dk <= dq).astype(bf)
    w_out_c = np.ascontiguousarray(w_out.astype(bf))
    bo = np.ascontiguousarray(b_out.reshape(1, D).astype(bf))

    in_maps = []
    for c in range(NCORES):
        h0, h1 = 2 * c, 2 * c + 1

        def wcol(h, o):
            return w_qkv[:, 192 * h + o:192 * h + o + 64]

        def bcol(h, o):
            return b_qkv[192 * h + o:192 * h + o + 64]

        wq = np.concatenate([wcol(h0, 0), wcol(h1, 0)], axis=1) * np.float32(0.125)
        wk = np.concatenate([wcol(h0, 64), wcol(h1, 64)], axis=1)
        wv = np.concatenate([wcol(h0, 128), wcol(h1, 128)], axis=1)
        wc = np.ascontiguousarray(
            np.concatenate([wq, wk, wv], axis=1).astype(bf))
        bq = np.concatenate([bcol(h0, 0), bcol(h1, 0)]) * np.float32(0.125)
        bk = np.concatenate([bcol(h0, 64), bcol(h1, 64)])
        bv = np.concatenate([bcol(h0, 128), bcol(h1, 128)])
        bc = np.ascontiguousarray(
            np.stack([bq, bk, bv], axis=1).astype(np.float32))  # (128, 3)

        m = {"xT": xT, "wqkv": wc, "bqkv": bc, "wout": w_out_c, "bout": bo,
             "ident": const_ident}
        if variant == "causal":
            m["maskz"] = const_maskz
        if variant == "general":
            m["maskT"] = maskT
        in_maps.append(m)

    nc = _build(variant, exp_bias=exp_bias)
    trace = os.environ.get("SMSA_TRACE", "0") == "1"
    res = bass_utils.run_bass_kernel_spmd(
        nc, in_maps, core_ids=list(range(NCORES)), trace=trace)
    LAST_EXEC_NS = res.exec_time_ns
    LAST_RESULTS = res

    parts = [res.results[c]["out"] for c in range(NCORES)]
    out = np.concatenate(parts, axis=0).reshape(B, S, D)
    return np.ascontiguousarray(out.astype(np.float32, copy=False))


# revision 17
# speedup vs baseline: 1.0858x; 1.0858x over previous
"""Multi-head self-attention block on Trainium2, 8-core SPMD.

Problem (fixed shapes): x(2,2048,1024), causal-additive mask(2,2048,2048),
w_qkv(1024,3072), b_qkv(3072), w_out(1024,1024), b_out(1024).
out = MHSA(x) with H=16 heads, head_dim=64.

Sharding strategy:
  - QKV projection + attention: tensor-parallel over heads (2 heads/core).
    Each core computes Q^T,K^T,V for its 2 heads over all 4096 tokens.
  - All matmul operands are bf16 (inputs converted on host); PSUM
    accumulation stays fp32, so softmax denominators are exact sums of
    bf16-rounded exp terms. Scores are computed transposed ([keys, q])
    so softmax needs no transposes: exp on ScalarE over [128, 1024]
    pairs of key-blocks (halves ACTIVATE dispatch overhead), the
    denominator comes from an all-ones column appended to V (fused into
    the AV matmul), normalization is DVE reciprocal + GpSimd
    partition_broadcast + one DVE multiply.
  - Softmax skips max-subtraction: scores here are O(10) and exp() runs
    in fp32; masked lanes are exactly zero, matching exp(-1e9)==0.
  - Attention runs as two head-passes; each pass ends in a bf16
    AllToAll of its 64 attention features, so the first collective
    hides under the second pass's compute.
  - After the AllToAlls each core holds all 1024 attention features for
    its 512 tokens and runs the out projection (+bias via a rank-1
    matmul) for just those rows. Host concatenates the 8 row-blocks.
"""

import os
import sys
from contextlib import ExitStack

if "/opt/trn_rl_repo" not in sys.path:
    sys.path.insert(0, "/opt/trn_rl_repo")

import numpy as np

import concourse.mybir as mybir
import concourse.tile as tile
from concourse import bacc, bass_utils

B, S, D, H, HD = 2, 2048, 1024, 16, 64
NCORES = 8
SL = B * S            # 4096 tokens total
QC = 512              # q-chunk / moving free dim
KC = 128              # k-chunk (partition dim)
NQ = S // QC          # 4 q-chunks per batch
NK = S // KC          # 16 k-chunks per batch
NT = SL // QC         # 8 token chunks
DK = D // 128         # 8 contraction chunks of the model dim
VW = 2 * (HD + 1)     # 130: V-natural block width (2 heads x (64 V + ones col))

f32 = mybir.dt.float32
bf16 = mybir.dt.bfloat16
FX = mybir.ActivationFunctionType
ALU = mybir.AluOpType

LAST_EXEC_NS = None   # HW exec time (ns) of the last traced run
LAST_RESULTS = None


def _build(variant, exp_bias=0.0):
    """Emit the SPMD program. variant: 'causal' | 'dense' | 'general'."""
    assert variant in ("causal", "dense", "general")
    nc = bacc.Bacc("TRN2", target_bir_lowering=False, debug=False,
                   num_devices=NCORES)

    xT_d = nc.dram_tensor("xT", [D, SL], bf16, kind="ExternalInput")
    wqkv_d = nc.dram_tensor("wqkv", [D, 384], bf16, kind="ExternalInput")
    bqkv_d = nc.dram_tensor("bqkv", [128, 3], f32, kind="ExternalInput")
    wout_d = nc.dram_tensor("wout", [D, D], bf16, kind="ExternalInput")
    bout_d = nc.dram_tensor("bout", [1, D], bf16, kind="ExternalInput")
    ident_d = nc.dram_tensor("ident", [128, 128], bf16, kind="ExternalInput")
    if variant == "causal":
        # 0/1 multiplicative causal mask for the 4 diagonal sub-blocks,
        # applied to exp(scores) (exp(s-1e9)==0 == exp(s)*0).
        maskz_d = nc.dram_tensor("maskz", [128, 4 * QC], bf16, kind="ExternalInput")
    if variant == "general":
        maskT_d = nc.dram_tensor("maskT", [B, S, S], f32, kind="ExternalInput")
    out_d = nc.dram_tensor("out", [QC, D], f32, kind="ExternalOutput")

    with tile.TileContext(nc) as tc:
        with ExitStack() as stack:
            ep = stack.enter_context
            cpool = ep(tc.tile_pool(name="consts", bufs=1))
            big = ep(tc.tile_pool(name="big", bufs=1))
            xpool = ep(tc.tile_pool(name="xts", bufs=16))
            vpool = ep(tc.tile_pool(name="vstg", bufs=2))
            epool = ep(tc.tile_pool(name="epool", bufs=4))
            mpool = ep(tc.tile_pool(name="mpool", bufs=4))
            rpool = ep(tc.tile_pool(name="rpool", bufs=8))
            bcpool = ep(tc.tile_pool(name="bcpool", bufs=4))
            apool = ep(tc.tile_pool(name="apool", bufs=6))
            ppool = ep(tc.tile_pool(name="ppool", bufs=16))
            opool = ep(tc.tile_pool(name="opool", bufs=2))
            dram = ep(tc.tile_pool(name="dram", bufs=1, space="DRAM"))
            psmm = ep(tc.tile_pool(name="psmm", bufs=2, space="PSUM"))
            pssc = ep(tc.tile_pool(name="pssc", bufs=2, space="PSUM"))
            psav = ep(tc.tile_pool(name="psav", bufs=2, space="PSUM"))

            # ---------------- constants / resident tensors ----------------
            # w chunks are fetched interleaved with the first x chunks (see
            # qkv_gen) so the first matmul waits on just two transfers.
            w_sb = big.tile([128, DK * 384], bf16, name="w_sb")
            bq_sb = cpool.tile([128, 3], f32, name="bq_sb")
            ident = cpool.tile([128, 128], bf16, name="ident")
            if variant == "causal":
                maskz_sb = cpool.tile([128, 4 * QC], bf16, name="maskz_sb")
            ones_bf = cpool.tile([1, 128], bf16, name="ones_bf")
            nc.vector.memset(ones_bf[:], 1.0)

            wo_sb = big.tile([128, DK * D], bf16, name="wo_sb")
            bo_sb = cpool.tile([1, D], bf16, name="bo_sb")

            qT = big.tile([128, SL], bf16, name="qT")
            kT = big.tile([128, SL], bf16, name="kT")
            vn = big.tile([128, B * NK * VW], bf16, name="vn")
            # ones columns for the softmax denominator live at 64 + 65*j
            vn_ones = vn[:].rearrange("p (b c) -> p b c", c=HD + 1)[:, :, 64:65]
            nc.vector.memset(vn_ones, 1.0)

            a2a_in = [dram.tile([NCORES, HD, QC], bf16, name=f"a2a_in{h}")
                      for h in range(2)]
            a2a_out = [dram.tile([NCORES, HD, QC], bf16, name=f"a2a_out{h}")
                       for h in range(2)]

            # ---------------- phase 1: QKV projection (generator) ----------
            def qkv_gen(t):
                xts = []
                for dk in range(DK):
                    if t == 0:
                        nc.sync.dma_start(
                            w_sb[:, 384 * dk:384 * (dk + 1)],
                            wqkv_d.ap()[128 * dk:128 * (dk + 1), :])
                    xt = xpool.tile([128, QC], bf16, name=f"xt{t}_{dk}", tag="xt")
                    nc.sync.dma_start(
                        xt[:], xT_d.ap()[128 * dk:128 * (dk + 1),
                                         QC * t:QC * (t + 1)])
                    xts.append(xt)
                if t == 0:
                    nc.sync.dma_start(bq_sb[:], bqkv_d.ap())
                    nc.sync.dma_start(ident[:], ident_d.ap())
                    if variant == "causal":
                        nc.sync.dma_start(maskz_sb[:], maskz_d.ap())
                for m in range(3):
                    ps = psmm.tile([128, QC], f32, name=f"qkv{t}_{m}", tag="mm")
                    for dk in range(DK):
                        c0 = 384 * dk + 128 * m
                        nc.tensor.matmul(ps[:],
                                         w_sb[:, c0:c0 + 128],
                                         xts[dk][:],
                                         start=(dk == 0), stop=(dk == DK - 1))
                    bias_ap = bq_sb[:, m:m + 1]
                    if m == 0:
                        nc.vector.tensor_scalar_add(
                            out=qT[:, QC * t:QC * (t + 1)], in0=ps[:], scalar1=bias_ap)
                    elif m == 1:
                        nc.vector.tensor_scalar_add(
                            out=kT[:, QC * t:QC * (t + 1)], in0=ps[:], scalar1=bias_ap)
                    else:
                        vst = vpool.tile([128, QC], bf16, name=f"vst{t}", tag="vst")
                        nc.vector.tensor_scalar_add(out=vst[:], in0=ps[:], scalar1=bias_ap)
                        for ci in range(4):
                            gi = 4 * t + ci
                            trp = psmm.tile([128, 128], bf16, name=f"tr{gi}", tag="mm")
                            nc.tensor.transpose(trp[:], vst[:, 128 * ci:128 * (ci + 1)],
                                                ident[:])
                            nc.vector.tensor_copy(
                                out=vn[:, VW * gi:VW * gi + 64], in_=trp[:, 0:64])
                            nc.vector.tensor_copy(
                                out=vn[:, VW * gi + 65:VW * gi + 129], in_=trp[:, 64:128])
                    yield

            # ---------------- phase 2: attention (generator, one head) -----
            # pend holds at most one pair awaiting its AV matmuls so the AV
            # for pair p issues only after exp(p+1) is queued (PE never
            # waits on ScalarE back-to-back).
            pend = []

            def drain_pend():
                e, av, gis, h, st, sp_, fin = pend.pop(0)
                for z, gi in enumerate(gis):
                    nc.tensor.matmul(av[:],
                                     vn[:, VW * gi + 65 * h:VW * gi + 65 * h + 65],
                                     e[:, QC * z:QC * (z + 1)],
                                     start=(st and z == 0), stop=(sp_ and z == 1),
                                     skip_group_check=True)
                if fin is not None:
                    fin()

            def attn_gen(h, b, j):
                n_i = 4 * (j + 1) if variant == "causal" else NK
                npair = n_i // 2
                q0 = S * b + QC * j
                h0 = 64 * h
                av = psav.tile([65, QC], f32, name=f"av{h}_{b}_{j}", tag="av")

                def finalize():
                    dd = rpool.tile([1, QC], f32, name=f"dd{h}_{b}_{j}", tag="dd")
                    nc.vector.tensor_copy(out=dd[:], in_=av[64:65, :])
                    rr = rpool.tile([1, QC], f32, name=f"rr{h}_{b}_{j}", tag="rr")
                    nc.vector.reciprocal_approx_fast(out=rr[:], in_=dd[:])
                    bc = bcpool.tile([64, QC], f32, name=f"bc{h}_{b}_{j}", tag="bc")
                    nc.gpsimd.partition_broadcast(bc[:], rr[:], channels=64)
                    att = apool.tile([64, QC], bf16, name=f"att{h}_{b}_{j}", tag="att")
                    nc.vector.tensor_tensor(out=att[:], in0=av[0:64, :], in1=bc[:],
                                            op=ALU.mult)
                    nc.sync.dma_start(a2a_in[h][NQ * b + j], att[:])

                for p in range(npair):
                    i0, i1 = 2 * p, 2 * p + 1
                    gi0, gi1 = NK * b + i0, NK * b + i1
                    k0 = S * b + KC * i0
                    sp = pssc.tile([128, 2 * QC], f32, name=f"s{h}_{b}_{j}_{p}",
                                   tag="sc")
                    nc.tensor.matmul(sp[:, 0:QC], kT[h0:h0 + 64, k0:k0 + KC],
                                     qT[h0:h0 + 64, q0:q0 + QC],
                                     start=True, stop=True)
                    nc.tensor.matmul(sp[:, QC:2 * QC],
                                     kT[h0:h0 + 64, k0 + KC:k0 + 2 * KC],
                                     qT[h0:h0 + 64, q0:q0 + QC],
                                     start=True, stop=True)
                    if variant == "general":
                        mt = mpool.tile([128, 2 * QC], f32,
                                        name=f"mt{h}_{b}_{j}_{p}", tag="mt")
                        for z, ii in enumerate((i0, i1)):
                            nc.sync.dma_start(
                                mt[:, QC * z:QC * (z + 1)],
                                maskT_d.ap()[b, KC * ii:KC * (ii + 1),
                                             QC * j:QC * (j + 1)])
                        nc.vector.tensor_tensor(out=sp[:], in0=sp[:], in1=mt[:],
                                                op=ALU.add)
                    e = epool.tile([128, 2 * QC], bf16, name=f"e{h}_{b}_{j}_{p}",
                                   tag="e")
                    nc.scalar.activation(out=e[:], in_=sp[:], func=FX.Exp,
                                         bias=exp_bias)
                    if variant == "causal" and p >= npair - 2:
                        dp = p - (npair - 2)  # 0 or 1 -> mask cols m=0,1 / 2,3
                        nc.vector.tensor_tensor(
                            out=e[:], in0=e[:],
                            in1=maskz_sb[:, 2 * QC * dp:2 * QC * (dp + 1)],
                            op=ALU.mult)
                    pend.append((e, av, (gi0, gi1), h,
                                 p == 0, p == npair - 1,
                                 finalize if p == npair - 1 else None))
                    if len(pend) > 1:
                        drain_pend()
                    yield

            def run_all(gens):
                gens = list(gens)
                while gens:
                    nxt = []
                    for g in gens:
                        try:
                            next(g)
                            nxt.append(g)
                        except StopIteration:
                            pass
                    gens = nxt

            # ----- interleave qkv t-chunks with head-0 attention -----------
            blocks = [(b, j) for b in range(B) for j in range(NQ)]
            run_all([qkv_gen(0)])
            # out-projection weights trickle in behind the x chunks
            for dk in range(DK):
                nc.sync.dma_start(wo_sb[:, D * dk:D * (dk + 1)],
                                  wout_d.ap()[128 * dk:128 * (dk + 1), :])
            nc.sync.dma_start(bo_sb[:], bout_d.ap())
            for t in range(1, NT):
                b, j = blocks[t - 1]
                run_all([qkv_gen(t), attn_gen(0, b, j)])
            run_all([attn_gen(0, *blocks[NT - 1])])
            while pend:
                drain_pend()

            # ---------------- phase 3: AllToAll h0 + head-1 pass -----------
            nc.gpsimd.collective_compute(
                "AllToAll", ALU.bypass,
                replica_groups=[list(range(NCORES))],
                ins=[a2a_in[0].opt()], outs=[a2a_out[0].opt()])

            for b, j in blocks:
                run_all([attn_gen(1, b, j)])
            while pend:
                drain_pend()

            nc.gpsimd.collective_compute(
                "AllToAll", ALU.bypass,
                replica_groups=[list(range(NCORES))],
                ins=[a2a_in[1].opt()], outs=[a2a_out[1].opt()])

            # ---------------- phase 4: out projection ----------------------
            # all received-attention tiles load up front (they fire the
            # moment the AllToAll lands), then the matmuls stream.
            atw = []
            for dk in range(DK):
                at = ppool.tile([128, QC], bf16, name=f"atw{dk}", tag="at")
                for h in range(2):
                    nc.sync.dma_start(at[64 * h:64 * (h + 1), :],
                                      a2a_out[h][dk])
                atw.append(at)
            for qsub in range(4):
                for dc in range(2):
                    ps = psmm.tile([128, QC], f32, name=f"op{qsub}_{dc}", tag="mm")
                    for dk in range(DK):
                        c0 = D * dk + QC * dc
                        nc.tensor.matmul(ps[:],
                                         atw[dk][:, 128 * qsub:128 * (qsub + 1)],
                                         wo_sb[:, c0:c0 + QC],
                                         start=(dk == 0), stop=False)
                    nc.tensor.matmul(ps[:], ones_bf[:],
                                     bo_sb[0:1, QC * dc:QC * (dc + 1)],
                                     start=False, stop=True)
                    osb = opool.tile([128, QC], f32, name=f"osb{qsub}_{dc}", tag="osb")
                    nc.vector.tensor_copy(out=osb[:], in_=ps[:])
                    nc.sync.dma_start(
                        out_d.ap()[128 * qsub:128 * (qsub + 1),
                                   QC * dc:QC * (dc + 1)], osb[:])

    nc.finalize()
    return nc


def _detect_variant(mask):
    if not mask.any():
        return "dense"
    tri = np.where(np.tril(np.ones((S, S), dtype=bool)),
                   np.float32(0.0), np.float32(-1e9)).astype(np.float32)
    for b in range(B):
        if not np.array_equal(mask[b], tri):
            return "general"
    return "causal"


def kernel(**inputs):
    global LAST_EXEC_NS, LAST_RESULTS
    import ml_dtypes
    bf = ml_dtypes.bfloat16

    x = np.ascontiguousarray(np.asarray(inputs["x"], dtype=np.float32))
    mask = np.asarray(inputs["mask"], dtype=np.float32)
    w_qkv = np.asarray(inputs["w_qkv"], dtype=np.float32)
    b_qkv = np.asarray(inputs["b_qkv"], dtype=np.float32)
    w_out = np.ascontiguousarray(np.asarray(inputs["w_out"], dtype=np.float32))
    b_out = np.asarray(inputs["b_out"], dtype=np.float32)

    variant = _detect_variant(mask)

    exp_bias = 0.0
    maskT = None
    if variant in ("general", "dense"):
        # guard exp() against overflow: bound max score via norms; any
        # needed shift is folded into the (transposed) additive mask.
        xf = x.reshape(SL, D)
        qkv = xf @ w_qkv + b_qkv
        qkv = qkv.reshape(SL, H, 3 * HD)
        qn = np.linalg.norm(qkv[:, :, :HD], axis=2).max()
        kn = np.linalg.norm(qkv[:, :, HD:2 * HD], axis=2).max()
        mmax = 0.0 if variant == "dense" else max(0.0, float(np.nanmax(mask)))
        bound = qn * kn / np.sqrt(HD) + mmax
        shift = min(0.0, 60.0 - bound)
        if variant == "dense":
            exp_bias = shift
        if variant == "general":
            maskT = np.ascontiguousarray(
                mask.transpose(0, 2, 1) + np.float32(shift))

    xT = np.ascontiguousarray(x.reshape(SL, D).T.astype(bf))
    const_ident = np.eye(128, dtype=bf)
    const_maskz = None
    if variant == "causal":
        const_maskz = np.zeros((128, 4 * QC), dtype=bf)
        for m in range(4):
            dk = np.arange(128)[:, None]
            dq = np.arange(QC)[None, :]
            const_maskz[:, QC * m:QC * (m + 1)] = (
                128 * m + dk <= dq).astype(bf)
    w_out_c = np.ascontiguousarray(w_out.astype(bf))
    bo = np.ascontiguousarray(b_out.reshape(1, D).astype(bf))

    in_maps = []
    for c in range(NCORES):
        h0, h1 = 2 * c, 2 * c + 1

        def wcol(h, o):
            return w_qkv[:, 192 * h + o:192 * h + o + 64]

        def bcol(h, o):
            return b_qkv[192 * h + o:192 * h + o + 64]

        wq = np.concatenate([wcol(h0, 0), wcol(h1, 0)], axis=1) * np.float32(0.125)
        wk = np.concatenate([wcol(h0, 64), wcol(h1, 64)], axis=1)
        wv = np.concatenate([wcol(h0, 128), wcol(h1, 128)], axis=1)
        wc = np.ascontiguousarray(
            np.concatenate([wq, wk, wv], axis=1).astype(bf))
        bq = np.concatenate([bcol(h0, 0), bcol(h1, 0)]) * np.float32(0.125)
        bk = np.concatenate([bcol(h0, 64), bcol(h1, 64)])
        bv = np.concatenate([bcol(h0, 128), bcol(h1, 128)])
        bc = np.ascontiguousarray(
            np.stack([bq, bk, bv], axis=1).astype(np.float32))  # (128, 3)

        m = {"xT": xT, "wqkv": wc, "bqkv": bc, "wout": w_out_c, "bout": bo,
             "ident": const_ident}
        if variant == "causal":
            m["maskz"] = const_maskz
        if variant == "general":
            m["maskT"] = maskT
        in_maps.append(m)

    nc = _build(variant, exp_bias=exp_bias)
    trace = os.environ.get("SMSA_TRACE", "0") == "1"
    res = bass_utils.run_bass_kernel_spmd(
        nc, in_maps, core_ids=list(range(NCORES)), trace=trace)
    LAST_EXEC_NS = res.exec_time_ns
    LAST_RESULTS = res

    parts = [res.results[c]["out"] for c in range(NCORES)]
    out = np.concatenate(parts, axis=0).reshape(B, S, D)
    return np.ascontiguousarray(out.astype(np.float32, copy=False))
